# revision 1
# baseline (speedup 1.0000x reference)
"""Single-head causal self-attention on 8 TRN2 NeuronCores.

Problem: B=8, T=2048, C=1024 fp32.
  q = x @ Wq.T + bq ; k = x @ Wk.T + bk ; v = x @ Wv.T + bv
  att = softmax(causal_mask(q @ k.T / sqrt(C)))
  out = att @ v

Sharding: data-parallel over batch — core b owns batch element b. No
collectives. Host pre-transposes x (-> [C, T]) and the weights (-> [C, C],
contraction-major) and casts matmul operands to bf16 so every TensorEngine
matmul runs at the 1-cycle/row bf16 rate. Softmax skips the max-subtraction
pass (logits here are ~N(0, 0.41^2); exp cannot overflow) which is
mathematically identical to the reference's stabilized softmax.

All DRAM inputs are laid out host-side so that every DMA moves one long
contiguous run per partition (descriptor-efficient), and x^T arrives in four
t-chunks so the first projection matmul only waits for ~1.25MB of DMA.

The v-projection bias passes through the attention average unchanged
(attention weights sum to 1), so bv is added once at the end instead of
being materialized into v.
"""

import numpy as np
import ml_dtypes

B, T, C = 8, 2048, 1024
P = 128            # partitions
C8 = C // P        # contraction chunks (8)
F8 = C // P        # feature chunks (8)
NT = T // P        # 16 query blocks of 128
ST = 512           # t-tile width for projections
SW = 256           # s-tile width for the score matrix (finer = less causal waste)
NST = T // ST      # 4 t-chunks across full T
FT = C // 512      # 2 output-feature tiles of 512
SCALE = 1.0 / np.sqrt(C)

BF16 = ml_dtypes.bfloat16


def build_nc():
    import concourse.tile as tile
    from concourse import bacc, mybir

    f32 = mybir.dt.float32
    bf16 = mybir.dt.bfloat16

    nc = bacc.Bacc()

    xt = nc.declare_dram_parameter("xt", [P, NST, C8 * ST], bf16, isOutput=False)
    wqt = nc.declare_dram_parameter("wqt", [F8, P, C8 * P], bf16, isOutput=False)
    wkt = nc.declare_dram_parameter("wkt", [F8, P, C8 * P], bf16, isOutput=False)
    wvt = nc.declare_dram_parameter("wvt", [P, C8 * C], bf16, isOutput=False)
    bk2 = nc.declare_dram_parameter("bk2", [P, F8], f32, isOutput=False)
    m2 = nc.declare_dram_parameter("m2", [P, C8], bf16, isOutput=False)
    ones1 = nc.declare_dram_parameter("ones1", [1, P], bf16, isOutput=False)
    bvb = nc.declare_dram_parameter("bvb", [P, C], f32, isOutput=False)
    masks = nc.declare_dram_parameter("masks", [P, 2 * SW], bf16, isOutput=False)
    ident = nc.declare_dram_parameter("ident", [P, P], bf16, isOutput=False)
    out = nc.declare_dram_parameter("out", [T, C], f32, isOutput=True)

    with tile.TileContext(nc) as tc:
        import contextlib
        ctx = contextlib.ExitStack()
        with ctx:
            consts = ctx.enter_context(tc.tile_pool(name="consts", bufs=1))
            wstream = ctx.enter_context(tc.tile_pool(name="wstream", bufs=2))
            work = ctx.enter_context(tc.tile_pool(name="work", bufs=1))
            ppool = ctx.enter_context(tc.tile_pool(name="ppool", bufs=6))
            ptpool = ctx.enter_context(tc.tile_pool(name="ptpool", bufs=4))
            lpool = ctx.enter_context(tc.tile_pool(name="lpool", bufs=3))
            opool = ctx.enter_context(tc.tile_pool(name="opool", bufs=4))
            psum = ctx.enter_context(tc.tile_pool(name="psum", bufs=1, space="PSUM"))

            # tiny bias tables first (they gate the first ACT copies)
            bk_sb = consts.tile([P, F8], f32, tag="bk")
            nc.sync.dma_start(out=bk_sb, in_=bk2[:, :])
            m2_sb = consts.tile([P, C8], bf16, tag="m2")
            nc.sync.dma_start(out=m2_sb, in_=m2[:, :])
            ones_sb = consts.tile([1, P], bf16, tag="ones1")
            nc.sync.dma_start(out=ones_sb, in_=ones1[:, :])

            # x^T in four t-chunks: one contiguous 8KB run per partition
            # each. All startup DMAs issue serially from the sync sequencer
            # (~0.6us each) — that serialization naturally staggers the
            # transfers so the first-needed chunks aren't bandwidth-starved
            # by later ones (measured faster than any multi-engine or
            # single-big-DMA variant).
            xt_sb = work.tile([P, NST, C8 * ST], bf16, tag="xt")
            for tc_ in range(NST):
                nc.sync.dma_start(out=xt_sb[:, tc_, :], in_=xt[:, tc_, :])
            xt_v = xt_sb.rearrange("p tc (c8 t) -> p tc c8 t", t=ST)

            kt_sb = work.tile([P, F8, T], bf16, tag="kt")
            v_sb = work.tile([P, NT, C], bf16, tag="v")

            # ---- phase 1: projections ----
            # q^T and k^T in feature-major layout [f, t]; v in token-major [s, f].
            for which, w_dram, bias_sb, dst, scale in (
                ("k", wkt, bk_sb, kt_sb, 1.0),
            ):
                for f8 in range(F8):
                    w_tile = wstream.tile([P, C8 * P], bf16, name="w_tile",
                                          tag="w_tile")
                    nc.sync.dma_start(out=w_tile, in_=w_dram[f8, :, :])
                    for tt in range(NST):
                        ps = psum.tile([P, ST], f32, tag="ps_a", bufs=4)
                        for c8 in range(C8):
                            nc.tensor.matmul(
                                ps,
                                w_tile[:, c8 * P:(c8 + 1) * P],
                                xt_v[:, tt, c8, :],
                                start=(c8 == 0),
                                stop=(c8 == C8 - 1),
                            )
                        nc.scalar.activation(
                            out=dst[:, f8, tt * ST:(tt + 1) * ST],
                            in_=ps,
                            func=mybir.ActivationFunctionType.Identity,
                            bias=bias_sb[:, f8:f8 + 1],
                            scale=scale,
                        )

            # v-projection weights stream in while k is being computed
            wv_sb = work.tile([P, C8 * C], bf16, tag="wv")
            for ch in range(2):
                half = C8 * C // 2
                nc.sync.dma_start(
                    out=wv_sb[:, ch * half:(ch + 1) * half],
                    in_=wvt[:, ch * half:(ch + 1) * half],
                )
            wv_v = wv_sb.rearrange("p (c8 f) -> p c8 f", f=C)

            for s16 in range(NT):
                for ft in range(FT):
                    ps = psum.tile([P, ST], f32, tag="ps_a", bufs=4)
                    for c8 in range(C8):
                        nc.tensor.matmul(
                            ps,
                            xt_v[:, s16 // 4, c8,
                                 (s16 % 4) * P:(s16 % 4 + 1) * P],
                            wv_v[:, c8, ft * 512:(ft + 1) * 512],
                            start=(c8 == 0),
                            stop=(c8 == C8 - 1),
                        )
                    nc.vector.tensor_copy(
                        out=v_sb[:, s16, ft * 512:(ft + 1) * 512], in_=ps
                    )

            # rank-1 softmax correction: w[s] = (bq^T Wk x^T)[s] / sqrt(C),
            # g = exp(w), G = broadcast of g to all 128 partitions. The other
            # bias cross-terms are row-constant in the logits and cancel in
            # softmax.
            g_sb = consts.tile([1, T], bf16, tag="g_sb")
            for st4 in range(NST):
                ps_g = psum.tile([P, ST], f32, tag="ps_b", bufs=2, name="ps_g")
                for c8 in range(C8):
                    nc.tensor.matmul(
                        ps_g[0:1, :],
                        m2_sb[:, c8:c8 + 1],
                        xt_v[:, st4, c8, :],
                        start=(c8 == 0),
                        stop=(c8 == C8 - 1),
                    )
                nc.scalar.activation(
                    out=g_sb[:, st4 * ST:(st4 + 1) * ST], in_=ps_g[0:1, :],
                    func=mybir.ActivationFunctionType.Exp,
                )
            G_sb = consts.tile([P, T], bf16, tag="G_sb")
            for st4 in range(NST):
                ps_G = psum.tile([P, ST], f32, tag="ps_b", bufs=2, name="ps_G")
                nc.tensor.matmul(
                    ps_G, ones_sb, g_sb[:, st4 * ST:(st4 + 1) * ST],
                    start=True, stop=True,
                )
                nc.vector.tensor_copy(
                    out=G_sb[:, st4 * ST:(st4 + 1) * ST], in_=ps_G
                )

            # phase-2 constants (needed from block 0 of attention)
            masks_sb = consts.tile([P, 2 * SW], bf16, tag="masks")
            nc.sync.dma_start(out=masks_sb, in_=masks[:, :])
            masks_v = masks_sb.rearrange("p (r s) -> p r s", s=SW)
            ident_sb = consts.tile([P, P], bf16, tag="ident")
            nc.sync.dma_start(out=ident_sb, in_=ident[:, :])
            bvb_sb = consts.tile([P, C], f32, tag="bvb")
            nc.sync.dma_start(out=bvb_sb, in_=bvb[:, :])

            # ---- phase 2: attention, one 128-row query block at a time ----
            for i in range(NT):
                n_chunks = i + 1                      # valid 128-wide s-chunks
                n_stiles = (n_chunks + 1) // 2        # 256-wide s-tiles
                r = i % 2                             # diagonal position in tile

                p_tiles = []
                lpart = lpool.tile([P, 8], mybir.dt.float32, name="lpart",
                                   tag="lpart")
                for j in range(n_stiles):
                    # even-i blocks have only 128 valid columns in the
                    # diagonal tile — compute just those
                    w = P if (j == n_stiles - 1 and r == 0) else SW
                    ps_s = psum.tile([P, SW], mybir.dt.float32, tag="ps_a",
                                     bufs=4, name="ps_s")
                    for f8 in range(F8):
                        nc.tensor.matmul(
                            ps_s[:, :w],
                            xt_v[:, i // 4, f8, (i % 4) * P:(i % 4 + 1) * P],
                            kt_sb[:, f8, j * SW:j * SW + w],
                            start=(f8 == 0),
                            stop=(f8 == F8 - 1),
                        )
                    p_sb = ppool.tile([P, SW], bf16, name="p_sb", tag="p_sb",
                                      bufs=8)
                    nc.scalar.activation(
                        out=p_sb[:, :w], in_=ps_s[:, :w],
                        func=mybir.ActivationFunctionType.Exp,
                    )
                    nc.vector.tensor_mul(p_sb[:, :w], p_sb[:, :w],
                                         G_sb[:, j * SW:j * SW + w])
                    if j == n_stiles - 1:
                        nc.vector.tensor_mul(p_sb[:, :w], p_sb[:, :w],
                                             masks_v[:, r, :w])
                    nc.vector.reduce_sum(
                        out=lpart[:, j:j + 1], in_=p_sb[:, :w],
                        axis=mybir.AxisListType.X,
                    )
                    p_tiles.append(p_sb)

                l_sum = lpool.tile([P, 1], mybir.dt.float32, name="l_sum",
                                   tag="l_sum")
                nc.vector.reduce_sum(
                    out=l_sum, in_=lpart[:, :n_stiles],
                    axis=mybir.AxisListType.X,
                )
                rl = lpool.tile([P, 1], mybir.dt.float32, name="rl", tag="rl")
                nc.vector.reciprocal(out=rl, in_=l_sum)

                ps_o = [
                    psum.tile([P, 512], mybir.dt.float32, tag="ps_c", bufs=2,
                              name="ps_o")
                    for _ in range(FT)
                ]
                for k in range(n_chunks):
                    pt_ps = psum.tile([P, P], bf16, tag="ps_b",
                                      bufs=2, name="pt_ps")
                    nc.tensor.transpose(
                        pt_ps,
                        p_tiles[k // 2][:, (k % 2) * P:(k % 2 + 1) * P],
                        ident_sb,
                    )
                    pt_sb = ptpool.tile([P, P], bf16, name="pt_sb", tag="pt_sb",
                                        bufs=4)
                    nc.vector.tensor_copy(out=pt_sb, in_=pt_ps)
                    for ft in range(FT):
                        nc.tensor.matmul(
                            ps_o[ft],
                            pt_sb,
                            v_sb[:, k, ft * 512:(ft + 1) * 512],
                            start=(k == 0),
                            stop=(k == n_chunks - 1),
                        )

                for ft in range(FT):
                    o_sb = opool.tile([P, 512], mybir.dt.float32, name="o_sb",
                                      tag="o_sb", bufs=4)
                    nc.scalar.activation(
                        out=o_sb, in_=ps_o[ft],
                        func=mybir.ActivationFunctionType.Copy,
                        scale=rl,
                    )
                    nc.vector.tensor_add(
                        out=o_sb, in0=o_sb, in1=bvb_sb[:, ft * 512:(ft + 1) * 512]
                    )
                    nc.sync.dma_start(
                        out=out[i * P:(i + 1) * P, ft * 512:(ft + 1) * 512],
                        in_=o_sb,
                    )

    nc.finalize()
    return nc


def make_in_maps(x, Wq, bq, Wk, bk, Wv, bv):
    """Host-side prep: per-core shards + replicated constants, all laid out
    partition-major so each DMA is one contiguous run per partition."""
    x = np.asarray(x, dtype=np.float32)
    wqt = np.ascontiguousarray(np.asarray(Wq, np.float32).T).astype(BF16)
    wkt = np.ascontiguousarray(np.asarray(Wk, np.float32).T).astype(BF16)
    wvt = np.ascontiguousarray(np.asarray(Wv, np.float32).T).astype(BF16)

    # [c, f] -> [f8, p, c8*fc] so each per-f8 DMA is contiguous per partition
    def wq_layout(wt):
        return np.ascontiguousarray(
            wt.reshape(C8, P, F8, P).transpose(2, 1, 0, 3).reshape(F8, P, C8 * P)
        )

    wq4 = wq_layout(wqt)  # unused by the kernel but kept for the wqt param
    wk32 = np.asarray(Wk, np.float32)
    wq32 = np.asarray(Wq, np.float32)
    m_mat = ((wk32.T @ wq32) * SCALE).astype(BF16)   # [c_in', c_out]
    wk4 = wq_layout(m_mat)
    m2v = ((wk32.T @ np.asarray(bq, np.float32)) * SCALE).astype(BF16)
    m2h = np.ascontiguousarray(m2v.reshape(C8, P).T)
    ones1 = np.ones((1, P), dtype=BF16)
    # [c, f] -> [p, c8*f]
    wv4 = np.ascontiguousarray(
        wvt.reshape(C8, P, C).transpose(1, 0, 2).reshape(P, C8 * C)
    )

    bk2 = np.zeros((P, F8), dtype=np.float32)
    bvb = np.tile(np.asarray(bv, np.float32)[None, :], (P, 1))

    masks = np.zeros((2, P, SW), dtype=np.float32)
    for rr in range(2):
        for tl in range(P):
            masks[rr, tl, : P * rr + tl + 1] = 1.0
    masks = np.ascontiguousarray(
        masks.astype(BF16).transpose(1, 0, 2).reshape(P, 2 * SW)
    )
    ident = np.eye(P, dtype=np.float32).astype(BF16)

    in_maps = []
    for b in range(B):
        xtb = np.ascontiguousarray(x[b].T).astype(BF16)
        # [c, t] -> [p, tc, c8*t]
        xt4 = np.ascontiguousarray(
            xtb.reshape(C8, P, NST, ST).transpose(1, 2, 0, 3)
            .reshape(P, NST, C8 * ST)
        )
        in_maps.append({
            "xt": xt4, "wqt": wq4, "wkt": wk4, "wvt": wv4,
            "bk2": bk2, "bvb": bvb, "m2": m2h, "ones1": ones1,
            "masks": masks, "ident": ident,
        })
    return in_maps


_CACHED_NC = None


def kernel(x, Wq, bq, Wk, bk, Wv, bv):
    global _CACHED_NC
    from concourse.bass_utils import run_bass_kernel_spmd

    if _CACHED_NC is None:
        _CACHED_NC = build_nc()
    in_maps = make_in_maps(x, Wq, bq, Wk, bk, Wv, bv)
    res = run_bass_kernel_spmd(_CACHED_NC, in_maps, core_ids=list(range(B)))
    return np.stack([res.results[b]["out"] for b in range(B)]).astype(np.float32)



# revision 4
# speedup vs baseline: 1.1385x; 1.1385x over previous
"""Single-head causal self-attention on 8 TRN2 NeuronCores (v2).

Problem: B=8, T=2048, C=1024 fp32.
  q = x @ Wq.T + bq ; k = x @ Wk.T + bk ; v = x @ Wv.T + bv
  att = softmax(causal_mask(q @ k.T / sqrt(C)))
  out = att @ v

Sharding: data-parallel over batch — core b owns batch element b, no
collectives.

Math restructuring (relative to the straightforward formulation):
  - Q and K projections fuse into ONE projection: dropping softmax-row-
    constant terms, scores == (x M + b~) @ x^T with M = Wq^T Wk / sqrt(C)
    and b~ = bq Wk / sqrt(C). So only two TxCxC projections total (q~, v)
    plus the two T^2 C/2 causal attention matmuls.
  - The q~ projection runs in fp8 (e4m3) with perf_mode=DoubleRow (2 fp8
    weights per PE cell -> 2x MAC rate). M and x are quantized HOST-side;
    M is pre-scaled by 512 so its ~4e-4-magnitude entries stay out of the
    fp8 subnormal range, and the 1/512 descale rides the psum->SBUF
    activation for free. Measured end-to-end rel err ~1.7e-2 (vs 3.5e-3
    all-bf16) -- inside the 2e-2 budget. Set FP8_PROJ=False for the
    all-bf16 fallback.
  - Scores are computed TRANSPOSED (p^T[s, t] = exp(scores)^T): the exp'd
    tile then feeds att@V directly as the matmul stationary operand, so
    the per-tile PE transposes + vector copies of the p matrix disappear.
  - softmax denominators l[t] come from a 1-column matmul against a ones
    vector that reuses the p^T stationary already in the PE array.
  - v's bias passes through the attention average unchanged (weights sum
    to 1), so bv is added once at the end.
  - No max-subtraction in softmax: logits are ~N(0, 0.41^2), exp cannot
    overflow; mathematically identical to the stabilized softmax.
"""

import numpy as np
import ml_dtypes

B, T, C = 8, 2048, 1024
P = 128              # partitions
C8 = C // P          # 128-deep contraction chunks (8)
K4 = C // 256        # 256-deep DoubleRow chunks (4)
NT = T // P          # 16 token blocks of 128
SW = 256             # phase-2 superblock width (2 token blocks)
NSB = T // SW        # 8 superblocks
TCH = 512            # phase-1 t-chunk width
NTCH = T // TCH      # 4
SCALE = 1.0 / np.sqrt(C)
SM = 512.0           # host pre-scale on M before fp8 quantization

FP8_PROJ = True

BF16 = ml_dtypes.bfloat16
FP8 = ml_dtypes.float8_e4m3


def build_nc():
    import contextlib
    import concourse.tile as tile
    from concourse import bacc, mybir

    f32 = mybir.dt.float32
    bf16 = mybir.dt.bfloat16
    fp8 = mybir.dt.float8e4
    DR = mybir.MatmulPerfMode.DoubleRow

    nc = bacc.Bacc()

    if FP8_PROJ:
        m8 = nc.declare_dram_parameter("m8", [P, K4 * 2 * C], fp8, isOutput=False)
        x8 = nc.declare_dram_parameter("x8", [P, NTCH, K4 * 2 * TCH], fp8,
                                       isOutput=False)
    else:
        mt = nc.declare_dram_parameter("mt", [P, C8 * C], bf16, isOutput=False)
    xt = nc.declare_dram_parameter("xt", [P, C8, T], bf16, isOutput=False)
    wvt = nc.declare_dram_parameter("wvt", [P, C8 * C], bf16, isOutput=False)
    btb = nc.declare_dram_parameter("btb", [P, C8], f32, isOutput=False)
    bvb = nc.declare_dram_parameter("bvb", [P, C], f32, isOutput=False)
    triu = nc.declare_dram_parameter("triu", [P, P], bf16, isOutput=False)
    ones1 = nc.declare_dram_parameter("ones1", [P, 1], bf16, isOutput=False)
    out = nc.declare_dram_parameter("out", [T, C], f32, isOutput=True)

    with tile.TileContext(nc) as tc:
        ctx = contextlib.ExitStack()
        with ctx:
            consts = ctx.enter_context(tc.tile_pool(name="consts", bufs=1))
            work = ctx.enter_context(tc.tile_pool(name="work", bufs=1))
            p8pool = ctx.enter_context(tc.tile_pool(name="p8pool", bufs=6))
            lpool = ctx.enter_context(tc.tile_pool(name="lpool", bufs=4))
            opool = ctx.enter_context(tc.tile_pool(name="opool", bufs=4))
            psum = ctx.enter_context(tc.tile_pool(name="psum", bufs=1,
                                                  space="PSUM"))

            # ---- input DMAs, ordered so the q~ projection can start first
            btb_sb = consts.tile([P, C8], f32, tag="btb")
            nc.sync.dma_start(out=btb_sb, in_=btb[:, :])
            if FP8_PROJ:
                m8_sb = work.tile([P, K4 * 2 * C], fp8, tag="m8")
                nc.sync.dma_start(out=m8_sb, in_=m8[:, :])
                m8_v = m8_sb.rearrange("p (k i f) -> p k i f", i=2, f=C)
                x8_sb = work.tile([P, NTCH, K4 * 2 * TCH], fp8, tag="x8")
                for tch in range(NTCH):
                    nc.sync.dma_start(out=x8_sb[:, tch, :], in_=x8[:, tch, :])
                x8_v = x8_sb.rearrange("p tc (k i u) -> p tc k i u", i=2, u=TCH)
            else:
                mt_sb = work.tile([P, C8 * C], bf16, tag="mt")
                nc.sync.dma_start(out=mt_sb, in_=mt[:, :])
                mt_v = mt_sb.rearrange("p (c8 f) -> p c8 f", f=C)

            xt_sb = work.tile([P, C8, T], bf16, tag="xt")
            for c8 in range(C8):
                nc.sync.dma_start(out=xt_sb[:, c8, :], in_=xt[:, c8, :])
            wv_sb = work.tile([P, C8 * C], bf16, tag="wv")
            for h in range(2):
                half = C8 * C // 2
                nc.sync.dma_start(out=wv_sb[:, h * half:(h + 1) * half],
                                  in_=wvt[:, h * half:(h + 1) * half])
            wv_v = wv_sb.rearrange("p (c8 f) -> p c8 f", f=C)

            bvb_sb = consts.tile([P, C], f32, tag="bvb")
            nc.sync.dma_start(out=bvb_sb, in_=bvb[:, :])
            triu_sb = consts.tile([P, P], bf16, tag="triu")
            nc.sync.dma_start(out=triu_sb, in_=triu[:, :])
            ones_sb = consts.tile([P, 1], bf16, tag="ones1")
            nc.sync.dma_start(out=ones_sb, in_=ones1[:, :])

            qt_sb = work.tile([P, C8, T], bf16, tag="qt")
            v_sb = work.tile([P, NT, C], bf16, tag="v")

            # ---- phase 1a: fused q~ projection: q~^T[f, t] in SBUF bf16
            for tch in range(NTCH):
                for fb2 in range(C8):
                    ps = psum.tile([P, TCH], f32, tag="ps_o", bufs=4,
                                   name="ps_proj")
                    if FP8_PROJ:
                        for K in range(K4):
                            nc.tensor.matmul(
                                ps,
                                m8_v[:, K, :, fb2 * P:(fb2 + 1) * P],
                                x8_v[:, tch, K, :, :],
                                start=(K == 0),
                                stop=(K == K4 - 1),
                                perf_mode=DR,
                            )
                    else:
                        for c8 in range(C8):
                            nc.tensor.matmul(
                                ps,
                                mt_v[:, c8, fb2 * P:(fb2 + 1) * P],
                                xt_sb[:, c8, tch * TCH:(tch + 1) * TCH],
                                start=(c8 == 0),
                                stop=(c8 == C8 - 1),
                            )
                    nc.scalar.activation(
                        out=qt_sb[:, fb2, tch * TCH:(tch + 1) * TCH],
                        in_=ps,
                        func=mybir.ActivationFunctionType.Identity,
                        bias=btb_sb[:, fb2:fb2 + 1],
                        scale=(1.0 / SM) if FP8_PROJ else 1.0,
                    )

            # ---- phase 1b: v projection (token-major v[s, f])
            for sb in range(NT):
                for ft in range(2):
                    ps = psum.tile([P, 512], f32, tag="ps_o", bufs=4,
                                   name="ps_v")
                    for c8 in range(C8):
                        nc.tensor.matmul(
                            ps,
                            xt_sb[:, c8, sb * P:(sb + 1) * P],
                            wv_v[:, c8, ft * 512:(ft + 1) * 512],
                            start=(c8 == 0),
                            stop=(c8 == C8 - 1),
                        )
                    nc.vector.tensor_copy(
                        out=v_sb[:, sb, ft * 512:(ft + 1) * 512], in_=ps
                    )

            # ---- phase 2: attention, one 256-token superblock at a time.
            # p^T[s, t] per 128-deep s-chunk; att@V consumes p^T chunks as
            # the stationary operand (no transposes). Software-pipelined:
            # att@V of chunk k-1 is emitted after scores of chunk k so the
            # PE never waits on the Act engine's exp.
            for j in range(NSB):
                nch = 2 * j + 2                # s-chunks 0 .. 2j+1
                b0, b1 = 2 * j, 2 * j + 1      # the two 128-token t-blocks
                t0 = j * SW

                ps_o = [
                    psum.tile([P, 512], f32, tag="ps_o", bufs=4,
                              name=f"ps_o{bi}{ft}")
                    for bi in range(2) for ft in range(2)
                ]
                ps_l = [
                    psum.tile([P, 1], f32, tag="ps_l", bufs=2, name=f"ps_l{bi}")
                    for bi in range(2)
                ]
                p8_tiles = [None] * nch

                def attv(k):
                    p8 = p8_tiles[k]
                    for bi, b in enumerate((b0, b1)):
                        if k > b:
                            continue           # dead block (s entirely > t)
                        for ft in range(2):
                            nc.tensor.matmul(
                                ps_o[2 * bi + ft],
                                p8[:, bi * P:(bi + 1) * P],
                                v_sb[:, k, ft * 512:(ft + 1) * 512],
                                start=(k == 0),
                                stop=(k == b),
                            )
                        nc.tensor.matmul(
                            ps_l[bi],
                            p8[:, bi * P:(bi + 1) * P],
                            ones_sb,
                            start=(k == 0),
                            stop=(k == b),
                        )

                for k in range(nch):
                    ps_s = psum.tile([P, SW], f32, tag="ps_s", bufs=2,
                                     name="ps_s")
                    for c8 in range(C8):
                        nc.tensor.matmul(
                            ps_s,
                            xt_sb[:, c8, k * P:(k + 1) * P],
                            qt_sb[:, c8, t0:t0 + SW],
                            start=(c8 == 0),
                            stop=(c8 == C8 - 1),
                        )
                    p8 = p8pool.tile([P, SW], bf16, tag="p8", name="p8")
                    nc.scalar.activation(
                        out=p8, in_=ps_s,
                        func=mybir.ActivationFunctionType.Exp,
                    )
                    r = k - 2 * j
                    if r >= 0:  # diagonal chunk: causal mask, multiplicative
                        nc.vector.tensor_mul(
                            p8[:, r * P:(r + 1) * P],
                            p8[:, r * P:(r + 1) * P],
                            triu_sb,
                        )
                    p8_tiles[k] = p8
                    if k >= 1:
                        attv(k - 1)
                attv(nch - 1)

                # epilogue: out = ps_o / l + bv
                for bi, b in enumerate((b0, b1)):
                    rl = lpool.tile([P, 1], f32, tag="rl", name="rl")
                    nc.vector.reciprocal(out=rl, in_=ps_l[bi])
                    for ft in range(2):
                        o_sb = opool.tile([P, 512], f32, tag="o_sb",
                                          name="o_sb")
                        nc.scalar.activation(
                            out=o_sb, in_=ps_o[2 * bi + ft],
                            func=mybir.ActivationFunctionType.Copy,
                            scale=rl,
                        )
                        nc.vector.tensor_add(
                            out=o_sb, in0=o_sb,
                            in1=bvb_sb[:, ft * 512:(ft + 1) * 512],
                        )
                        nc.sync.dma_start(
                            out=out[b * P:(b + 1) * P, ft * 512:(ft + 1) * 512],
                            in_=o_sb,
                        )

    nc.finalize()
    return nc


def make_in_maps(x, Wq, bq, Wk, bk, Wv, bv):
    """Host-side prep: fused-projection matrix, fp8 quantization, and
    partition-major layouts so every DMA is contiguous per partition."""
    x = np.asarray(x, np.float32)
    Wq = np.asarray(Wq, np.float32)
    Wk = np.asarray(Wk, np.float32)
    Wv = np.asarray(Wv, np.float32)
    bq = np.asarray(bq, np.float32)
    bv = np.asarray(bv, np.float32)

    M = (Wq.T @ Wk) * SCALE                      # [c, f]
    bt = (bq @ Wk) * SCALE                       # [f]

    common = {}
    if FP8_PROJ:
        # m8[p, K, i, f] = SM * M[K*256 + i*128 + p, f], quantized e4m3
        common["m8"] = np.ascontiguousarray(
            (M * SM).reshape(K4, 2, P, C).transpose(2, 0, 1, 3)
            .reshape(P, K4 * 2 * C)
        ).astype(FP8)
    else:
        common["mt"] = np.ascontiguousarray(
            M.reshape(C8, P, C).transpose(1, 0, 2).reshape(P, C8 * C)
        ).astype(BF16)
    # wv[p, c8, f] = Wv.T[c8*128 + p, f]
    common["wvt"] = np.ascontiguousarray(
        Wv.T.reshape(C8, P, C).transpose(1, 0, 2).reshape(P, C8 * C)
    ).astype(BF16)
    common["btb"] = np.ascontiguousarray(bt.reshape(C8, P).T)
    common["bvb"] = np.tile(bv[None, :], (P, 1))
    common["triu"] = np.triu(np.ones((P, P), np.float32)).astype(BF16)
    common["ones1"] = np.ones((P, 1), np.float32).astype(BF16)

    in_maps = []
    for b in range(B):
        xtb = np.ascontiguousarray(x[b].T)       # [C, T] fp32
        d = dict(common)
        d["xt"] = np.ascontiguousarray(
            xtb.reshape(C8, P, T).transpose(1, 0, 2)
        ).astype(BF16)
        if FP8_PROJ:
            # x8[p, tc, K, i, u] = x^T[K*256 + i*128 + p, tc*512 + u]
            d["x8"] = np.ascontiguousarray(
                xtb.reshape(K4, 2, P, NTCH, TCH).transpose(2, 3, 0, 1, 4)
                .reshape(P, NTCH, K4 * 2 * TCH)
            ).astype(FP8)
        in_maps.append(d)
    return in_maps


_CACHED_NC = None


def kernel(x, Wq, bq, Wk, bk, Wv, bv):
    global _CACHED_NC
    from concourse.bass_utils import run_bass_kernel_spmd

    if _CACHED_NC is None:
        _CACHED_NC = build_nc()
    in_maps = make_in_maps(x, Wq, bq, Wk, bk, Wv, bv)
    res = run_bass_kernel_spmd(_CACHED_NC, in_maps, core_ids=list(range(B)))
    return np.stack([res.results[b]["out"] for b in range(B)]).astype(np.float32)
